# revision 1
# baseline (speedup 1.0000x reference)
"""TAGConv×2 + PReLU + global_add_pool + linear head on 8 trn2 cores.

Strategy (all FP math on device except graph-structure preprocessing):
 - A_norm = D^-1/2 A D^-1/2 factorized into per-node scaling: propagation is
   h' = dinv ⊙ scatter_sum(gather(g)), with g = dinv ⊙ h. The gathered table
   G is always "g" (pre-scaled), so the scatter one-hot S is pure 0/1.
 - Nodes padded to 50176 and split into 8 contiguous shards (6272/core).
   Edges live on the core owning their dst, sorted/grouped by dst tile (64
   nodes). Per dst tile, edges split into lo-src (<25088) / hi-src halves so
   int16 dma_gather indices work; each half padded to a uniform tile count
   (T_LO/T_HI tiles of 128 edges). Pad edges have dstloc=999 -> zero one-hot.
 - Gather: batched dma_gather (NB tiles/call) from the DRAM table (AllGather
   output; hop 1 of layer 0 reads the host-supplied g0).
 - Scatter: per edge tile, S[e,j]=(dstloc[e]==j) via tensor_scalar is_equal
   against an iota row; matmul(psum[64f,512n] slice, lhsT=msg, rhs=S)
   accumulates feature-major h'.
 - Per 512-node group: hT_k table = psum ⊙ dinvT (for the k-hop W matmuls),
   gT = psum ⊙ dinv2T -> PE-transpose -> node-major g -> DMA -> AllGather.
 - Layer output: psum_out[n,o] = Σ_k hT_k^T @ W[k]; +b, PReLU=max(v,αv);
   layer 0 feeds g/hT0 for layer 1; layer 1 feeds the pooling matmul
   (one-hot of batch_ids) on z = h·Wout, then AllReduce + bout.
"""
import math
import os
import numpy as np

_TRUNC = os.environ.get("KTRUNC", "full")


def _build(inputs, repeat=1):
    import concourse.bass as bass
    import concourse.bacc as bacc
    import concourse.mybir as mybir
    import concourse.tile as tile
    from concourse.library_config import mlp
    from concourse.masks import make_identity
    from concourse.bass import _add_dep_helper

    x = np.asarray(inputs["x"], np.float32)
    edge_index = np.asarray(inputs["edge_index"], np.int64)
    batch_ids = np.asarray(inputs["batch_ids"], np.int64)
    W0 = np.asarray(inputs["W0"], np.float32)
    b0 = np.asarray(inputs["b0"], np.float32)
    W1 = np.asarray(inputs["W1"], np.float32)
    b1 = np.asarray(inputs["b1"], np.float32)
    alpha0 = float(np.asarray(inputs["alpha0"]).reshape(-1)[0])
    alpha1 = float(np.asarray(inputs["alpha1"]).reshape(-1)[0])
    Wout = np.asarray(inputs["Wout"], np.float32)
    bout = float(np.asarray(inputs["bout"]).reshape(-1)[0])

    N, D = x.shape
    H = W0.shape[2]
    K = W0.shape[0] - 1  # hops
    NG = 128  # num graphs
    NC = 8
    JT = 64  # dst tile width
    GRP = 8  # dst tiles per psum group (512 nodes)
    P = 128
    NB = 32  # gather tiles per dma_gather call

    NPC = int(math.ceil(N / (NC * P))) * P  # nodes per core, 128-aligned
    NPAD = NC * NPC
    NJ = NPC // JT             # dst tiles per core
    NBLK = NPC // P            # 128-node blocks per core
    NGRP = int(math.ceil(NJ / GRP))
    HALF = (NPAD // 2 + P - 1) // P * P  # lo/hi split point
    assert HALF < 32768 and NPAD - HALF < 32768

    src = edge_index[0].astype(np.int64)
    dst = edge_index[1].astype(np.int64)

    # ---- host graph preprocessing (indices only + degree norm) ----
    deg = np.bincount(dst, minlength=N).astype(np.float64)
    dinv = np.zeros(NPAD, np.float32)
    nz = deg > 0
    dinv[:N][nz] = (1.0 / np.sqrt(deg[nz])).astype(np.float32)
    dinv2 = dinv * dinv

    core_of = dst // NPC
    order = np.lexsort((src, dst))  # by dst, then src
    src_s, dst_s = src[order], dst[order]
    core_s = core_of[order]

    # per (core, dst tile): lo/hi edge lists
    lists = [[[None, None] for _ in range(NJ)] for _ in range(NC)]
    T_LO = 1
    T_HI = 1
    for r in range(NC):
        m = core_s == r
        sr, dr = src_s[m], dst_s[m]
        jt = (dr % NPC) // JT
        lo = sr < HALF
        for j in range(NJ):
            mj = jt == j
            s_lo = sr[mj & lo]
            d_lo = dr[mj & lo] % JT
            s_hi = sr[mj & ~lo] - HALF
            d_hi = dr[mj & ~lo] % JT
            lists[r][j][0] = (s_lo, d_lo)
            lists[r][j][1] = (s_hi, d_hi)
            T_LO = max(T_LO, (len(s_lo) + P - 1) // P)
            T_HI = max(T_HI, (len(s_hi) + P - 1) // P)

    NT = NJ * T_LO  # tiles per lo stream (hi: NJ*T_HI)
    NT_HI = NJ * T_HI
    assert 8 * T_LO <= 48 and 8 * T_HI <= 48, (T_LO, T_HI)
    # one gather call per (psum group, stream): jn*T tiles
    grp_jn = [min(GRP, NJ - g * GRP) for g in range(NGRP)]

    def pack_stream(r, half, T):
        """idx16 [ntiles*P], dstloc [P, ntiles] f32 for one core's stream."""
        ntiles = NJ * T
        idx = np.zeros(ntiles * P, np.int16)
        dl = np.full((P, ntiles), 999.0, np.float32)
        for j in range(NJ):
            s, d = lists[r][j][half]
            n = len(s)
            base = j * T * P
            idx[base : base + n] = s.astype(np.int16)
            t0 = j * T
            for u in range((n + P - 1) // P):
                seg = slice(u * P, min((u + 1) * P, n))
                cnt = seg.stop - seg.start
                dl[:cnt, t0 + u] = d[seg].astype(np.float32)
        return idx, dl

    def wrap_idx(idx, T):
        """[NJ*T*P] -> [P, NJ*T*8] int16, wrapped per call (= per group)."""
        cols = []
        t0 = 0
        for g in range(NGRP):
            nt = grp_jn[g] * T
            blk = idx[t0 * P : (t0 + nt) * P]
            t0 += nt
            w = np.zeros((16, nt * 8), np.int16)
            ii = np.arange(nt * P)
            w[ii % 16, ii // 16] = blk
            cols.append(np.tile(w, (8, 1)))
        return np.concatenate(cols, axis=1)

    # ---- per-core host inputs ----
    xpad = np.zeros((NPAD, D), np.float32)
    xpad[:N] = x
    g0 = xpad * dinv[:, None]
    bpad = np.full(NPAD, 999.0, np.float32)
    bpad[:N] = batch_ids.astype(np.float32)

    per_core = []
    for r in range(NC):
        idx_lo, dl_lo = pack_stream(r, 0, T_LO)
        idx_hi, dl_hi = pack_stream(r, 1, T_HI)
        sl = slice(r * NPC, (r + 1) * NPC)
        m = dict(
            g0=g0,
            idx_lo=wrap_idx(idx_lo, T_LO),
            idx_hi=wrap_idx(idx_hi, T_HI),
            dstloc=np.concatenate([dl_lo, dl_hi], axis=1),
            xT=np.ascontiguousarray(xpad[sl].T),
            dinvT=np.ascontiguousarray(np.tile(dinv[sl][None, :], (H, 1))),
            dinv_nm=np.ascontiguousarray(dinv[sl].reshape(NBLK, P).T),
            batch_nm=np.ascontiguousarray(bpad[sl].reshape(NBLK, P).T),
            iota64=np.tile(np.arange(JT, dtype=np.float32)[None, :], (P, 1)),
            iota128=np.tile(np.arange(NG, dtype=np.float32)[None, :], (P, 1)),
            W0=W0, W1=W1,
            b0b=np.tile(b0[None, :], (P, 1)),
            b1b=np.tile(b1[None, :], (P, 1)),
            woutb=np.tile(Wout[:, 0][None, :], (P, 1)),
        )
        per_core.append(m)

    # ---- device program ----
    f32 = mybir.dt.float32
    i16 = mybir.dt.int16
    nc = bacc.Bacc("TRN2", target_bir_lowering=False, debug=False, num_devices=NC,
                   dynamic_dma_scratch_size=18432)

    def ein(name, shape, dtype=f32):
        return nc.dram_tensor(name, shape, dtype, kind="ExternalInput")

    g0_e = ein("g0", [NPAD, D])
    idx_lo_e = ein("idx_lo", [P, NT * 8], i16)
    idx_hi_e = ein("idx_hi", [P, NT_HI * 8], i16)
    dstloc_e = ein("dstloc", [P, NT + NT_HI])
    xT_e = ein("xT", [H, NPC])
    dinvT_e = ein("dinvT", [H, NPC])
    dinv_nm_e = ein("dinv_nm", [P, NBLK])
    batch_nm_e = ein("batch_nm", [P, NBLK])
    iota64_e = ein("iota64", [P, JT])
    iota128_e = ein("iota128", [P, NG])
    W0_e = ein("W0", [K + 1, D, H])
    W1_e = ein("W1", [K + 1, H, H])
    b0b_e = ein("b0b", [P, H])
    b1b_e = ein("b1b", [P, H])
    woutb_e = ein("woutb", [P, H])
    out_e = nc.dram_tensor("out", [NG, 1], f32, kind="ExternalOutput")

    G_shared = nc.dram_tensor("G_shared", [NPAD, D], f32, addr_space="Shared")
    G_in = nc.dram_tensor("G_in", [NPC, D], f32)
    ar_in = nc.dram_tensor("ar_in", [NG, 1], f32)
    ar_out = nc.dram_tensor("ar_out", [NG, 1], f32, addr_space="Shared")
    RG = [list(range(NC))]

    with tile.TileContext(nc) as tc:
        with (
            tc.tile_pool(name="const", bufs=1) as cpool,
            tc.tile_pool(name="s", bufs=4) as s_pool,
            tc.tile_pool(name="work", bufs=3) as work_pool,
            tc.tile_pool(name="ps_sc", bufs=2, space="PSUM") as ps_sc,
            tc.tile_pool(name="ps_tr", bufs=2, space="PSUM") as ps_tr,
            tc.tile_pool(name="ps_w", bufs=2, space="PSUM") as ps_w,
            tc.tile_pool(name="ps_pool", bufs=2, space="PSUM") as ps_pool,
        ):
            lib_i = nc.gpsimd.load_library(mlp)
            nidx_sizes = sorted({jn * T * P for jn in set(grp_jn) for T in (T_LO, T_HI)})
            nidx_regs = {n: nc.gpsimd.to_reg(n) for n in nidx_sizes}

            # persistent SBUF
            hT = [nc.alloc_sbuf_tensor(f"hT{k}", [H, NPC], f32) for k in range(K + 1)]
            msgbuf = [nc.alloc_sbuf_tensor("msg0", [P, GRP * T_LO, D], f32),
                      nc.alloc_sbuf_tensor("msg1", [P, GRP * T_HI, D], f32)]
            dinvT = nc.alloc_sbuf_tensor("dinvT_sb", [H, NPC], f32)
            idx_lo_sb = nc.alloc_sbuf_tensor("idx_lo_sb", [P, NT * 8], i16)
            idx_hi_sb = nc.alloc_sbuf_tensor("idx_hi_sb", [P, NT_HI * 8], i16)
            dstloc_sb = nc.alloc_sbuf_tensor("dstloc_sb", [P, NT + NT_HI], f32)
            iota64 = cpool.tile([P, JT], f32)
            iota128 = cpool.tile([P, NG], f32)
            dinv_nm = cpool.tile([P, NBLK], f32)
            batch_nm = cpool.tile([P, NBLK], f32)
            Wsb = cpool.tile([D, 2, K + 1, H], f32)
            ident = cpool.tile([P, P], f32)
            ident64 = cpool.tile([H, H], f32)
            b0b = cpool.tile([P, H], f32)
            b1b = cpool.tile([P, H], f32)
            woutb = cpool.tile([P, H], f32)

            nc.sync.dma_start(out=hT[0].ap(), in_=xT_e[:])
            nc.sync.dma_start(out=dinvT.ap(), in_=dinvT_e[:])
            nc.sync.dma_start(out=idx_lo_sb.ap(), in_=idx_lo_e[:])
            nc.sync.dma_start(out=idx_hi_sb.ap(), in_=idx_hi_e[:])
            nc.sync.dma_start(out=dstloc_sb.ap(), in_=dstloc_e[:])
            nc.sync.dma_start(out=iota64[:], in_=iota64_e[:])
            nc.sync.dma_start(out=iota128[:], in_=iota128_e[:])
            nc.sync.dma_start(out=dinv_nm[:], in_=dinv_nm_e[:])
            nc.sync.dma_start(out=batch_nm[:], in_=batch_nm_e[:])
            nc.sync.dma_start(out=Wsb[:, 0], in_=W0_e.ap().rearrange("k i o -> i k o"))
            nc.sync.dma_start(out=Wsb[:, 1], in_=W1_e.ap().rearrange("k i o -> i k o"))
            make_identity(nc, ident[:])
            make_identity(nc, ident64[:])
            nc.sync.dma_start(out=b0b[:], in_=b0b_e[:])
            nc.sync.dma_start(out=b1b[:], in_=b1b_e[:])
            nc.sync.dma_start(out=woutb[:], in_=woutb_e[:])

            W8 = NB * 8  # idx columns per call

            def do_hop(kk, layer, first_hop):
                """one propagation: per-group gather calls + scatter."""
                src_tab = g0_e if (layer == 0 and first_hop) else G_shared
                lo_ap = src_tab[0:HALF, :]
                hi_ap = src_tab[HALF:NPAD, :]
                need_g = kk < K  # last hop of a layer: no G production here
                for grp in range(NGRP):
                    j0 = grp * GRP
                    jn = grp_jn[grp]
                    pw = jn * JT
                    # gather this group's lo + hi tiles (one call per stream)
                    for st, (T, idxsb, ap_) in enumerate(
                        ((T_LO, idx_lo_sb, lo_ap), (T_HI, idx_hi_sb, hi_ap))
                    ):
                        nt = jn * T
                        col = j0 * T * 8
                        gi = nc.gpsimd.dma_gather(
                            msgbuf[st].ap()[:, :nt],
                            ap_,
                            idxsb.ap()[:, col : col + nt * 8],
                            nt * P, nidx_regs[nt * P], D, single_packet=False,
                        )
                        _add_dep_helper(gi.ins, lib_i.ins, True, "lib first")
                    ps = ps_sc.tile([H, GRP * JT], f32)
                    for jj in range(jn):
                        j = j0 + jj
                        reg = ps[:, jj * JT : (jj + 1) * JT]
                        for u in range(T_LO):
                            t = j * T_LO + u
                            S = s_pool.tile([P, JT], f32)
                            nc.vector.tensor_scalar(
                                S[:], iota64[:], dstloc_sb.ap()[:, t : t + 1],
                                None, mybir.AluOpType.is_equal,
                            )
                            nc.tensor.matmul(
                                reg, lhsT=msgbuf[0].ap()[:, jj * T_LO + u], rhs=S[:],
                                start=(u == 0), stop=False,
                            )
                        for u in range(T_HI):
                            t = j * T_HI + u
                            S = s_pool.tile([P, JT], f32)
                            nc.vector.tensor_scalar(
                                S[:], iota64[:],
                                dstloc_sb.ap()[:, NT + t : NT + t + 1],
                                None, mybir.AluOpType.is_equal,
                            )
                            nc.tensor.matmul(
                                reg, lhsT=msgbuf[1].ap()[:, jj * T_HI + u], rhs=S[:],
                                start=False, stop=(u == T_HI - 1),
                            )
                    cols = slice(j0 * JT, j0 * JT + pw)
                    nc.vector.tensor_tensor(
                        out=hT[kk].ap()[:, cols], in0=ps[:, :pw],
                        in1=dinvT.ap()[:, cols], op=mybir.AluOpType.mult,
                    )
                    if need_g:
                        gT = work_pool.tile([H, GRP * JT], f32, tag="gT")
                        nc.vector.tensor_tensor(
                            out=gT[:, :pw], in0=hT[kk].ap()[:, cols],
                            in1=dinvT.ap()[:, cols], op=mybir.AluOpType.mult,
                        )
                        gnm = work_pool.tile([P, GRP * JT // P, D], f32, tag="gnm")
                        nchunk = pw // P
                        for cch in range(nchunk):
                            pst = ps_tr.tile([P, H], f32, tag="pst")
                            nc.tensor.transpose(
                                out=pst[:], in_=gT[:, cch * P : (cch + 1) * P],
                                identity=ident64[:],
                            )
                            nc.vector.tensor_copy(out=gnm[:, cch], in_=pst[:])
                        rows = G_in[j0 * JT : j0 * JT + pw, :]
                        nc.sync.dma_start(
                            out=rows.rearrange("(c p) f -> p c f", p=P),
                            in_=gnm[:, :nchunk],
                        )
                if need_g:
                    nc.gpsimd.collective_compute(
                        "AllGather", mybir.AluOpType.bypass,
                        replica_groups=RG, ins=[G_in[:]], outs=[G_shared[:]],
                    )

            def do_layer(layer, skip_pool=False):
                alpha = alpha0 if layer == 0 else alpha1
                bb = b0b if layer == 0 else b1b
                for k in range(1, K + 1):
                    do_hop(k, layer, first_hop=(k == 1))
                # W matmuls + bias + PReLU per 128-node block
                for nb_i in range(NBLK):
                    pso = ps_w.tile([P, H], f32)
                    for k in range(K + 1):
                        nc.tensor.matmul(
                            pso[:], lhsT=hT[k].ap()[:, nb_i * P : (nb_i + 1) * P],
                            rhs=Wsb[:, layer, k], start=(k == 0), stop=(k == K),
                        )
                    v = work_pool.tile([P, H], f32, tag="v")
                    nc.vector.tensor_tensor(
                        out=v[:], in0=pso[:], in1=bb[:], op=mybir.AluOpType.add
                    )
                    av = work_pool.tile([P, H], f32, tag="av")
                    nc.vector.tensor_scalar_mul(av[:], v[:], float(alpha))
                    hh = work_pool.tile([P, H], f32, tag="hh")
                    op = mybir.AluOpType.max if alpha <= 1.0 else mybir.AluOpType.min
                    nc.vector.tensor_tensor(out=hh[:], in0=v[:], in1=av[:], op=op)
                    if layer == 0:
                        gg = work_pool.tile([P, H], f32, tag="gg")
                        nc.vector.tensor_scalar(
                            gg[:], hh[:], dinv_nm[:, nb_i : nb_i + 1], None,
                            mybir.AluOpType.mult,
                        )
                        nc.sync.dma_start(
                            out=G_in[nb_i * P : (nb_i + 1) * P, :], in_=gg[:]
                        )
                        pst2 = ps_tr.tile([H, P], f32, tag="pst")
                        # hT0 for layer 1: transpose hh -> [H, P]
                        nc.tensor.transpose(out=pst2[:], in_=hh[:], identity=ident[:])
                        nc.vector.tensor_copy(
                            out=hT[0].ap()[:, nb_i * P : (nb_i + 1) * P],
                            in_=pst2[:],
                        )
                    elif skip_pool == "z":
                        junk = work_pool.tile([P, H], f32, tag="junk")
                        z = work_pool.tile([P, 1], f32, tag="z")
                        nc.vector.tensor_tensor(
                            out=junk[:], in0=hh[:], in1=woutb[:],
                            op=mybir.AluOpType.mult,
                        )
                        nc.vector.reduce_sum(z[:], junk[:], mybir.AxisListType.X)
                        if nb_i == NBLK - 1:
                            nc.sync.dma_start(out=out_e[:, 0:1], in_=z[:])
                    elif skip_pool:
                        if nb_i == NBLK - 1:
                            nc.sync.dma_start(out=out_e[:, 0:1], in_=hh[:, 0:1])
                    else:
                        junk = work_pool.tile([P, H], f32, tag="junk")
                        z = work_pool.tile([P, 1], f32, tag="z")
                        nc.vector.tensor_tensor(
                            out=junk[:], in0=hh[:], in1=woutb[:],
                            op=mybir.AluOpType.mult,
                        )
                        nc.vector.reduce_sum(z[:], junk[:], mybir.AxisListType.X)
                        Sb = s_pool.tile([P, NG], f32, tag="Sb")
                        nc.vector.tensor_scalar(
                            Sb[:], iota128[:], batch_nm[:, nb_i : nb_i + 1],
                            None, mybir.AluOpType.is_equal,
                        )
                        psp = ps_pool.tile([NG, 1], f32, tag="psp")
                        nc.tensor.matmul(psp[:], lhsT=Sb[:], rhs=z[:],
                                         start=True, stop=True)
                        if nb_i == 0:
                            nc.vector.tensor_copy(out=pool_acc[:], in_=psp[:])
                        else:
                            nc.vector.tensor_tensor(
                                out=pool_acc[:], in0=pool_acc[:], in1=psp[:],
                                op=mybir.AluOpType.add,
                            )
                if layer == 0:
                    nc.gpsimd.collective_compute(
                        "AllGather", mybir.AluOpType.bypass,
                        replica_groups=RG, ins=[G_in[:]], outs=[G_shared[:]],
                    )

            pool_acc = cpool.tile([NG, 1], f32)
            if _TRUNC == "h1":
                do_hop(1, 0, first_hop=True)
            elif _TRUNC == "h12":
                do_hop(1, 0, first_hop=True)
                do_hop(2, 0, first_hop=False)
            elif _TRUNC == "L0":
                do_layer(0)
            elif _TRUNC == "L1":
                do_layer(0)
                do_layer(1)
            elif _TRUNC == "L1x":
                do_layer(0)
                do_layer(1, skip_pool=True)
            elif _TRUNC == "L1z":
                do_layer(0)
                do_layer(1, skip_pool="z")
            if _TRUNC != "full":
                dbg = work_pool.tile([H, 1], f32, tag="dbg")
                nc.vector.tensor_copy(out=dbg[:], in_=hT[1].ap()[:, 0:1])
                nc.sync.dma_start(out=out_e[0:H, :], in_=dbg[:])
            for _rep in range(repeat if _TRUNC == "full" else 0):
                do_layer(0)
                do_layer(1)

                nc.sync.dma_start(out=ar_in[:], in_=pool_acc[:])
                nc.gpsimd.collective_compute(
                    "AllReduce", mybir.AluOpType.add,
                    replica_groups=RG, ins=[ar_in[:]], outs=[ar_out[:]],
                )
                res = work_pool.tile([NG, 1], f32, tag="res")
                nc.sync.dma_start(out=res[:], in_=ar_out[:])
                nc.vector.tensor_scalar_add(res[:], res[:], float(bout))
                nc.sync.dma_start(out=out_e[:], in_=res[:])

    nc.compile()
    return nc, per_core


def simulate_debug(inputs, n_sim_cores=8):
    import concourse.bass_interp as bass_interp

    nc, per_core = _build(inputs, repeat=1)
    sim = bass_interp.MultiCoreSim(nc, n_sim_cores)
    for i in range(n_sim_cores):
        for k, v in per_core[i].items():
            sim.cores[i].tensor(k)[:] = v
    sim.simulate()
    return sim.cores[0].tensor("out").copy()


def kernel(**inputs):
    from concourse.bass_utils import run_bass_kernel_spmd

    nc, per_core = _build(inputs, repeat=1)
    results = run_bass_kernel_spmd(nc, per_core, list(range(8)))
    return results.results[0]["out"].astype(np.float32)


def estimate_hw_time_ns(inputs, r_hi=3, n_rep=8):
    import time
    from concourse.bass_utils import run_bass_kernel_spmd

    walls = {}
    for r in (1, r_hi):
        nc, per_core = _build(inputs, repeat=r)
        ws = []
        run_bass_kernel_spmd(nc, per_core, list(range(8)))  # warm
        for _ in range(n_rep):
            t0 = time.time()
            run_bass_kernel_spmd(nc, per_core, list(range(8)))
            ws.append(time.time() - t0)
        walls[r] = min(ws)
    return (walls[r_hi] - walls[1]) / (r_hi - 1) * 1e9


if __name__ == "__main__":
    import jax
    import reference

    cpu = jax.devices("cpu")[0]
    with jax.default_device(cpu):
        ins = reference.setup_inputs()
        ins = {k: np.asarray(v) for k, v in ins.items()}
        exp = np.asarray(reference.reference(**ins))
    got = kernel(**ins)
    err = np.abs(got - exp).max() / (np.abs(exp).max() + 1e-12)
    print("rel err:", err)



# revision 8
# speedup vs baseline: 5.5788x; 5.5788x over previous
"""TAGConv x2 + PReLU + global_add_pool, instruction-minimal for trn2 x8.

Design (per core, feature-major [64 part, nodes]):
 - G table in DRAM: [NPAD, 128] bf16 rows (64 feats + 64 zero pad, 256B).
 - Hop: dma_gather(transpose=True) pulls edge messages as columns
   (features on partitions 0-63), in octet order (8 same-dst edges per
   octet, zero-padded via a guaranteed-zero position). gpsimd scatter_add
   (d=8) accumulates octets into tab[64, DH+1, 8]; each call has UNIQUE
   dst indices (one octet per dst per rank-call) so the Q7 RMW races on
   duplicate indices never occur. Fold 8 slots (3 strided adds using msgT
   as scratch) then scale by dinv -> hT_k.
 - G exchange: gT=hT_k*dinv -> dma_start_transpose -> node-major gnm ->
   DMA into AG_in rows (upper lanes pre-zeroed) -> AllGather -> G_shared.
 - W phase: 13 psum chunks x 4 matmuls (lhsT=W[k] 64x64) + Prelu act
   (bias as per-partition AP) writing next-layer h (bf16).
 - Pool: dma_start_transpose h1 -> [128,49,64]; 49 accumulating matmuls
   with host-built one-hot Bnm -> PT[64 feat, 128 graph] psum; 1 matmul
   with Wout -> z[1,128]; AllReduce; +bout.
Node id -> position permutation swaps 25087 <-> 50100 so position 25087
(lo half) is a guaranteed-zero row for gather padding (dinv=0 there).
"""
import os
import numpy as np
import ml_dtypes

_V2T = os.environ.get("V2T", "full")
GSUB = int(os.environ.get("GSUB", "8192"))  # max edges per dma_gather

N, NPAD, NC = 50000, 50176, 8
NPC = NPAD // NC            # 6272
DH = NPC // 2               # 3136
K = 3
NGR = 128
HALF = NPAD // 2            # 25088
SWAP_A, SWAP_B = 25087, 50100
CH_OCT = 2304               # octets per gather chunk (x16); CH_OCT*8 >= 5*DH
CH_E = CH_OCT * 8           # 22016 edge columns in msgT
NBLK = NPC // 128           # 49


def _wrap16(idx):
    n = len(idx)
    w = np.zeros((16, (n + 15) // 16), np.int16)
    jj = np.arange(n)
    w[jj % 16, jj // 16] = idx
    return np.tile(w, (8, 1))[:128]


def _host_prep(inputs):
    x = np.asarray(inputs["x"], np.float32)
    edge_index = np.asarray(inputs["edge_index"], np.int64)
    batch_ids = np.asarray(inputs["batch_ids"], np.int64)

    src, dst = edge_index[0], edge_index[1]
    deg = np.bincount(dst, minlength=N).astype(np.float64)
    dinv_n = np.zeros(N, np.float32)
    nz = deg > 0
    dinv_n[nz] = (1.0 / np.sqrt(deg[nz])).astype(np.float32)

    # node -> position permutation
    nodes = np.arange(N)
    pn = np.where(nodes == SWAP_A, SWAP_B, nodes)  # SWAP_B>=N so no clash
    dinv_p = np.zeros(NPAD, np.float32)
    x_p = np.zeros((NPAD, x.shape[1]), np.float32)
    batch_p = np.full(NPAD, NGR, np.int64)
    dinv_p[pn] = dinv_n
    x_p[pn] = x
    batch_p[pn] = batch_ids

    ps = np.where(src == SWAP_A, SWAP_B, src)
    pd = np.where(dst == SWAP_A, SWAP_B, dst)

    # per core, per (dh, sh): octet arrays + per-rank scatter idx
    # core_data[r] = list over (dh, sh) of (list_of_rank_octets, list_of_rank_sidx)
    core_data = []
    for r in range(NC):
        m = (pd >= r * NPC) & (pd < (r + 1) * NPC)
        eps, epd = ps[m], pd[m] % NPC
        segs = []
        for dh in range(2):
            for sh in range(2):
                mm = ((epd >= dh * DH) & (epd < (dh + 1) * DH)
                      & (eps >= sh * HALF) & (eps < (sh + 1) * HALF))
                s_ = eps[mm] - sh * HALF
                d_ = epd[mm] - dh * DH
                zr = np.int16(25087)  # zero position, relative (both halves)
                order = np.argsort(d_, kind="stable")
                s_, d_ = s_[order], d_[order]
                cnt = np.bincount(d_, minlength=DH)
                dstart = np.r_[0, np.cumsum(cnt)[:-1]]
                rank_oct, rank_sidx = [], []
                rk = 0
                while True:
                    sel = np.nonzero(cnt > 8 * rk)[0]
                    if len(sel) == 0:
                        break
                    octs = np.full((len(sel), 8), zr, np.int16)
                    for slot in range(8):
                        has = cnt[sel] > 8 * rk + slot
                        octs[has, slot] = s_[dstart[sel[has]] + 8 * rk + slot]
                    rank_oct.append(octs)
                    rank_sidx.append(sel.astype(np.int16))
                    rk += 1
                segs.append((rank_oct, rank_sidx))
        core_data.append(segs)

    # global schedule: per (seg, rank): n_oct = max over cores, rounded x16
    sched = []  # list of (seg_id, rank, n_oct)
    for seg_id in range(4):
        rmax = max(len(core_data[r][seg_id][0]) for r in range(NC))
        for rk in range(rmax):
            n = max((len(core_data[r][seg_id][0][rk])
                     if rk < len(core_data[r][seg_id][0]) else 0)
                    for r in range(NC))
            n = ((n + 15) // 16) * 16
            sched.append((seg_id, rk, n))

    # chunks: greedy within seg, splitting calls at x16 boundaries
    # chunk = (sh, [(call_id, off_oct, n_oct_sub)...])
    chunks = []
    cur = None
    cur_fill = 0
    for cid, (seg_id, rk, n) in enumerate(sched):
        sh = seg_id % 2
        off = 0
        while off < n:
            if cur is None or cur[0] != (seg_id // 2, sh) or cur_fill >= CH_OCT:
                cur = ((seg_id // 2, sh), [])
                chunks.append(cur)
                cur_fill = 0
            take = min(CH_OCT - cur_fill, n - off)
            cur[1].append((cid, off, take))
            cur_fill += take
            off += take
    return (x_p, dinv_p, batch_p, core_data, sched, chunks)


def _build(inputs, repeat=1):
    import concourse.bacc as bacc
    import concourse.mybir as mybir
    import concourse.tile as tile
    from concourse.library_config import mlp
    from concourse.bass import _add_dep_helper

    f32 = mybir.dt.float32
    bf16 = mybir.dt.bfloat16
    i16 = mybir.dt.int16

    W0 = np.asarray(inputs["W0"], np.float32)
    b0 = np.asarray(inputs["b0"], np.float32)
    W1 = np.asarray(inputs["W1"], np.float32)
    b1 = np.asarray(inputs["b1"], np.float32)
    alphas = [float(np.asarray(inputs["alpha0"]).reshape(-1)[0]),
              float(np.asarray(inputs["alpha1"]).reshape(-1)[0])]
    Wout = np.asarray(inputs["Wout"], np.float32)
    bout = float(np.asarray(inputs["bout"]).reshape(-1)[0])

    x_p, dinv_p, batch_p, core_data, sched, chunks = _host_prep(inputs)

    g0_rows = np.zeros((NPAD, 128), np.float32)
    g0_rows[:, 0:64] = x_p * dinv_p[:, None]

    # per-core blobs following sched
    per_core = []
    for r in range(NC):
        g_parts, s_parts = [], []
        for seg_id, rk, n in sched:
            ro, rs = core_data[r][seg_id]
            if rk < len(ro):
                octs, sidx = ro[rk], rs[rk]
            else:
                octs = np.zeros((0, 8), np.int16)
                sidx = np.zeros(0, np.int16)
            pad = n - len(octs)
            octs = np.vstack([octs, np.full((pad, 8), 25087, np.int16)])
            sidx = np.r_[sidx, np.full(pad, DH, np.int16)]
            g_parts.append(octs.reshape(-1))
            s_parts.append(sidx)
        gblob = np.concatenate(g_parts)
        sblob = np.concatenate(s_parts)
        sl = slice(r * NPC, (r + 1) * NPC)
        xT = np.ascontiguousarray(x_p[sl].T)
        dinvT = np.tile(dinv_p[sl][None, :], (64, 1))
        bp = batch_p[sl].reshape(NBLK, 128)
        Bnm = np.zeros((128, NBLK, 128), np.float32)
        for b in range(NBLK):
            valid = bp[b] < NGR
            Bnm[np.arange(128)[valid], b, bp[b][valid]] = 1.0
        m = dict(
            g0=g0_rows.astype(ml_dtypes.bfloat16).view(np.int16),
            gidx=_wrap16(gblob),
            sidx=_wrap16(sblob),
            xT=xT.astype(ml_dtypes.bfloat16).view(np.int16),
            dinvT=dinvT.astype(ml_dtypes.bfloat16).view(np.int16),
            Wsb=np.ascontiguousarray(
                np.stack([W0, W1]).transpose(2, 0, 1, 3).reshape(64, 512)
            ).astype(ml_dtypes.bfloat16).view(np.int16),
            Woutb=Wout.astype(ml_dtypes.bfloat16).view(np.int16),
            bb=np.stack([b0, b1], 1).astype(np.float32),
            Bnm=np.ascontiguousarray(Bnm.reshape(128, NBLK * 128)
                                     ).astype(ml_dtypes.bfloat16).view(np.int16),
        )
        per_core.append(m)

    GW = per_core[0]["gidx"].shape[1]
    SW = per_core[0]["sidx"].shape[1]

    nc = bacc.Bacc("TRN2", target_bir_lowering=False, debug=False,
                   num_devices=NC, dynamic_dma_scratch_size=32768)

    def ein(name, shape, dtype=f32):
        return nc.dram_tensor(name, shape, dtype, kind="ExternalInput")

    g0_e = ein("g0", [NPAD, 128], i16)
    gidx_e = ein("gidx", [128, GW], i16)
    sidx_e = ein("sidx", [128, SW], i16)
    xT_e = ein("xT", [64, NPC], i16)
    dinvT_e = ein("dinvT", [64, NPC], i16)
    Wsb_e = ein("Wsb", [64, 8 * 64], i16)
    Wout_e = ein("Woutb", [64, 1], i16)
    bb_e = ein("bb", [64, 2])
    Bnm_e = ein("Bnm", [128, NBLK * 128], i16)
    out_e = nc.dram_tensor("out", [NGR, 1], f32, kind="ExternalOutput")

    G_shared = nc.dram_tensor("G_shared", [NPAD, 128], bf16, addr_space="Shared")
    AG_in = nc.dram_tensor("AG_in", [NPC, 128], bf16)
    ar_in = nc.dram_tensor("ar_in", [1, NGR], f32)
    ar_out = nc.dram_tensor("ar_out", [1, NGR], f32, addr_space="Shared")
    RG = [list(range(NC))]

    # per-call scatter idx offsets (in octets)
    soffs = np.r_[0, np.cumsum([n for _, _, n in sched])]

    with tile.TileContext(nc) as tc:
        with (
            tc.tile_pool(name="c", bufs=1) as cpool,
            tc.tile_pool(name="w", bufs=3) as wp,
            tc.tile_pool(name="pw", bufs=4, space="PSUM") as pw,
            tc.tile_pool(name="pp", bufs=1, space="PSUM") as pp,
        ):
            lib_i = nc.gpsimd.load_library(mlp)
            regs = {}

            def reg(n):
                if n not in regs:
                    regs[n] = nc.gpsimd.to_reg(n)
                return regs[n]

            msgT = nc.alloc_sbuf_tensor("msgT", [128, CH_E], bf16)
            tab = nc.alloc_sbuf_tensor("tab", [64, DH + 1, 8], bf16)
            hT = [nc.alloc_sbuf_tensor(f"hT{k}", [64, NPC], bf16)
                  for k in range(K + 1)]
            gT = nc.alloc_sbuf_tensor("gT", [64, NPC], bf16)
            gnm = nc.alloc_sbuf_tensor("gnm", [128, NBLK, 64], bf16)
            dinvT = nc.alloc_sbuf_tensor("dinvT_sb", [64, NPC], bf16)
            gidx = nc.alloc_sbuf_tensor("gidx_sb", [128, GW], i16)
            sidx = nc.alloc_sbuf_tensor("sidx_sb", [128, SW], i16)
            Wsb = cpool.tile([64, 8, 64], bf16)
            Woutb = cpool.tile([64, 1], bf16)
            bb = cpool.tile([64, 2], f32)
            PTsb = cpool.tile([64, NGR], bf16)

            nc.sync.dma_start(out=hT[0].ap(), in_=xT_e.ap().bitcast(bf16))
            nc.sync.dma_start(out=dinvT.ap(), in_=dinvT_e.ap().bitcast(bf16))
            nc.sync.dma_start(out=gidx.ap(), in_=gidx_e[:])
            nc.sync.dma_start(out=sidx.ap(), in_=sidx_e[:])
            nc.sync.dma_start(out=Wsb[:].rearrange("p a b -> p (a b)"),
                              in_=Wsb_e.ap().bitcast(bf16))
            nc.sync.dma_start(out=Woutb[:], in_=Wout_e.ap().bitcast(bf16))
            nc.sync.dma_start(out=bb[:], in_=bb_e[:])
            zq = msgT.ap()[:, 0 : NBLK * 64]
            nc.vector.memset(zq, 0.0)
            nc.sync.dma_start(
                out=AG_in.ap().rearrange("(c p) f -> p c f", p=128)[:, :, 64:128],
                in_=zq.rearrange("p (c f) -> p c f", f=64),
            )

            def do_hop(kk, first, gather_en=True, scatter_en=True):
                src_tab = g0_e.ap().bitcast(bf16) if first else G_shared.ap()
                # dh groups: chunks are ordered dh0 then dh1
                cur_dh = -1
                goff = 0  # edge offset into gidx blob
                for (dh, sh), parts in chunks:
                    if dh != cur_dh:
                        if cur_dh >= 0:
                            fold(cur_dh, kk)
                        nc.vector.memset(tab.ap().rearrange("p a b -> p (a b)"), 0.0)
                        cur_dh = dh
                    ne = sum(t * 8 for _, _, t in parts)
                    tab_in = (src_tab[0:HALF, :] if sh == 0
                              else src_tab[HALF:NPAD, :])
                    if gather_en:
                        gs = GSUB if GSUB else ne
                        for sub in range(0, ne, gs):
                            nsub = min(gs, ne - sub)
                            gi = nc.gpsimd.dma_gather(
                                msgT.ap()[:, sub : sub + nsub]
                                    .rearrange("p (one n) -> p one n", one=1),
                                tab_in,
                                gidx.ap()[:, (goff + sub) // 16
                                          : (goff + sub + nsub) // 16],
                                nsub, reg(nsub), 128,
                                transpose=True, single_packet=False,
                            )
                            _add_dep_helper(gi.ins, lib_i.ins, True, "lib first")
                    goff += ne
                    co = 0  # octet offset within chunk
                    for cid, off, t in parts:
                        if not scatter_en:
                            continue
                        so = soffs[cid] + off
                        sa = nc.gpsimd.scatter_add(
                            tab.ap(),
                            sidx.ap()[0:64, so // 16 : (so + t) // 16],
                            msgT.ap()[0:64, co * 8 : (co + t) * 8]
                                .rearrange("p (n d) -> p n d", d=8),
                            64, DH + 1, 8, t,
                        )
                        _add_dep_helper(sa.ins, lib_i.ins, True, "lib first")
                        co += t
                fold(cur_dh, kk)

            def fold(dh, kk):
                """tab[64, DH, 8] -> hT[kk][:, dh*DH:] = sum(slots)*dinv."""
                tv = tab.ap()[:, 0:DH, :]
                tA = msgT.ap()[0:64, 0 : DH * 4].rearrange(
                    "p (n d) -> p n d", d=4)
                tB = gT.ap()[:, 0 : DH * 2].rearrange(
                    "p (n d) -> p n d", d=2)
                tC = msgT.ap()[0:64, DH * 4 : DH * 5]
                nc.vector.tensor_tensor(out=tA, in0=tv[:, :, 0:4],
                                        in1=tv[:, :, 4:8],
                                        op=mybir.AluOpType.add)
                nc.vector.tensor_tensor(out=tB, in0=tA[:, :, 0:2],
                                        in1=tA[:, :, 2:4],
                                        op=mybir.AluOpType.add)
                nc.vector.tensor_tensor(out=tC.rearrange("p (n d) -> p n d", d=1),
                                        in0=tB[:, :, 0:1], in1=tB[:, :, 1:2],
                                        op=mybir.AluOpType.add)
                cols = slice(dh * DH, (dh + 1) * DH)
                nc.vector.tensor_tensor(out=hT[kk].ap()[:, cols], in0=tC,
                                        in1=dinvT.ap()[:, cols],
                                        op=mybir.AluOpType.mult)

            def exchange(src):
                """src [64, NPC] bf16 = g values -> AllGather into G_shared."""
                nc.sync.dma_start_transpose(gnm.ap(), src)
                nc.sync.dma_start(
                    out=AG_in.ap().rearrange("(c p) f -> p c f", p=128)[:, :, 0:64],
                    in_=gnm.ap(),
                )
                nc.gpsimd.collective_compute(
                    "AllGather", mybir.AluOpType.bypass,
                    replica_groups=RG, ins=[AG_in[:]], outs=[G_shared[:]],
                )

            def wphase(layer):
                for c in range(13):
                    c0 = c * 512
                    cw = min(512, NPC - c0)
                    ps = pw.tile([64, 512], f32)
                    for k in range(K + 1):
                        nc.tensor.matmul(
                            ps[:, 0:cw], lhsT=Wsb[:, layer * 4 + k, :],
                            rhs=hT[k].ap()[:, c0 : c0 + cw],
                            start=(k == 0), stop=(k == K),
                        )
                    nc.scalar.activation(
                        hT[0].ap()[:, c0 : c0 + cw], ps[:, 0:cw],
                        mybir.ActivationFunctionType.Prelu,
                        bias=bb[:, layer : layer + 1], scale=1.0,
                        alpha=alphas[layer],
                    )

            def dbg_out(src_bf16_col):
                d = wp.tile([64, 1], f32, tag="dbg")
                nc.vector.tensor_copy(out=d[:], in_=src_bf16_col)
                nc.sync.dma_start(out=out_e[0:64, :], in_=d[:])

            if _V2T == "hop":
                do_hop(1, True)
                dbg_out(hT[1].ap()[:, 0:1])
            elif _V2T == "gonly":
                do_hop(1, True, scatter_en=False)
                dbg_out(hT[1].ap()[:, 0:1])
            elif _V2T == "sonly":
                do_hop(1, True, gather_en=False)
                dbg_out(hT[1].ap()[:, 0:1])
            elif _V2T == "hopx":
                do_hop(1, True)
                nc.vector.tensor_tensor(out=gT.ap()[:, :], in0=hT[1].ap()[:, :],
                                        in1=dinvT.ap()[:, :],
                                        op=mybir.AluOpType.mult)
                exchange(gT.ap()[:, :])
                do_hop(2, False)
                dbg_out(hT[2].ap()[:, 0:1])
            elif _V2T == "wp0":
                do_hop(1, True)
                wphase(0)
                dbg_out(hT[0].ap()[:, 0:1])
            for _rep in range(repeat if _V2T == "full" else 0):
                for layer in range(2):
                    first = layer == 0
                    for k in range(1, K + 1):
                        do_hop(k, first and k == 1)
                        if k < K:
                            gcols = gT.ap()[:, :]
                            nc.vector.tensor_tensor(
                                out=gcols, in0=hT[k].ap()[:, :],
                                in1=dinvT.ap()[:, :], op=mybir.AluOpType.mult)
                            exchange(gcols)
                    wphase(layer)
                    if layer == 0:
                        nc.vector.tensor_tensor(
                            out=gT.ap()[:, :], in0=hT[0].ap()[:, :],
                            in1=dinvT.ap()[:, :], op=mybir.AluOpType.mult)
                        exchange(gT.ap()[:, :])

                # pooling: h1 = hT[0]
                Bnm = msgT.ap()[:, 0 : NBLK * 128].rearrange(
                    "p (a b) -> p a b", b=128)
                nc.sync.dma_start(out=msgT.ap()[:, 0 : NBLK * 128],
                                  in_=Bnm_e.ap().bitcast(bf16))
                h1nm = msgT.ap()[:, NBLK * 128 : NBLK * 192].rearrange(
                    "p (a b) -> p a b", b=64)
                nc.sync.dma_start_transpose(h1nm, hT[0].ap()[:, :])
                PT = pp.tile([64, NGR], f32)
                for b in range(NBLK):
                    nc.tensor.matmul(PT[:], lhsT=h1nm[:, b, :], rhs=Bnm[:, b, :],
                                     start=(b == 0), stop=(b == NBLK - 1))
                nc.vector.tensor_copy(out=PTsb[:], in_=PT[:])
                zps = pp.tile([1, NGR], f32, tag="zps")
                nc.tensor.matmul(zps[:], lhsT=Woutb[:], rhs=PTsb[:],
                                 start=True, stop=True)
                zsb = wp.tile([1, NGR], f32, tag="zsb")
                nc.vector.tensor_copy(out=zsb[:], in_=zps[:])
                nc.sync.dma_start(out=ar_in[:], in_=zsb[:])
                nc.gpsimd.collective_compute(
                    "AllReduce", mybir.AluOpType.add,
                    replica_groups=RG, ins=[ar_in[:]], outs=[ar_out[:]],
                )
                res = wp.tile([1, NGR], f32, tag="res")
                nc.sync.dma_start(out=res[:], in_=ar_out[:])
                nc.vector.tensor_scalar_add(res[:], res[:], float(bout))
                nc.sync.dma_start(out=out_e.ap().rearrange("g one -> one g"),
                                  in_=res[:])

    nc.compile()
    return nc, per_core


def kernel(**inputs):
    from concourse.bass_utils import run_bass_kernel_spmd

    nc, per_core = _build(inputs, repeat=1)
    results = run_bass_kernel_spmd(nc, per_core, list(range(8)))
    return results.results[0]["out"].astype(np.float32)


def estimate_hw_time_ns(inputs, r_hi=3, n_rep=8):
    import time
    from concourse.bass_utils import run_bass_kernel_spmd

    walls = {}
    for r in (1, r_hi):
        nc, per_core = _build(inputs, repeat=r)
        run_bass_kernel_spmd(nc, per_core, list(range(8)))  # warm
        ws = []
        for _ in range(n_rep):
            t0 = time.time()
            run_bass_kernel_spmd(nc, per_core, list(range(8)))
            ws.append(time.time() - t0)
        walls[r] = min(ws)
    return (walls[r_hi] - walls[1]) / (r_hi - 1) * 1e9


if __name__ == "__main__":
    import jax
    import reference

    cpu = jax.devices("cpu")[0]
    with jax.default_device(cpu):
        ins = {k: np.asarray(v) for k, v in reference.setup_inputs().items()}
        exp = np.asarray(reference.reference(**ins))
    got = kernel(**ins)
    err = np.abs(got - exp).max() / (np.abs(exp).max() + 1e-12)
    print("rel err:", err)


# revision 13
# speedup vs baseline: 9.5692x; 1.7153x over previous
"""TAGConv x2 + PReLU + global_add_pool, instruction-minimal for trn2 x8.

Design (per core, feature-major [64 part, nodes]):
 - G table in DRAM: [NPAD, 128] bf16 rows (64 feats + 64 zero pad, 256B).
 - Hop: dma_gather(transpose=True) pulls edge messages as columns
   (features on partitions 0-63), in octet order (8 same-dst edges per
   octet, zero-padded via a guaranteed-zero position). gpsimd scatter_add
   (d=8) accumulates octets into tab[64, DH+1, 8]; each call has UNIQUE
   dst indices (one octet per dst per rank-call) so the Q7 RMW races on
   duplicate indices never occur. Fold 8 slots (3 strided adds using msgT
   as scratch) then scale by dinv -> hT_k.
 - G exchange: gT=hT_k*dinv -> dma_start_transpose -> node-major gnm ->
   DMA into AG_in rows (upper lanes pre-zeroed) -> AllGather -> G_shared.
 - W phase: 13 psum chunks x 4 matmuls (lhsT=W[k] 64x64) + Prelu act
   (bias as per-partition AP) writing next-layer h (bf16).
 - Pool: dma_start_transpose h1 -> [128,49,64]; 49 accumulating matmuls
   with host-built one-hot Bnm -> PT[64 feat, 128 graph] psum; 1 matmul
   with Wout -> z[1,128]; AllReduce; +bout.
Node id -> position permutation swaps 25087 <-> 50100 so position 25087
(lo half) is a guaranteed-zero row for gather padding (dinv=0 there).
"""
import os
import numpy as np
import ml_dtypes

_V2T = os.environ.get("V2T", "full")
GSUB = int(os.environ.get("GSUB", "8192"))  # max edges per dma_gather

N, NPAD, NC = 50000, 50176, 8
NPC = NPAD // NC            # 6272
DH = NPC // 2               # 3136
K = 3
NGR = 128
HALF = NPAD // 2            # 25088
SWAP_A, SWAP_B = 25087, 50100
CH_OCT = 2048               # octets per gather chunk (x16); CH_OCT*8 >= 5*DH
CH_E = CH_OCT * 8           # 22016 edge columns in msgT
NBLK = NPC // 128           # 49


def _wrap16(idx):
    n = len(idx)
    w = np.zeros((16, (n + 15) // 16), np.int16)
    jj = np.arange(n)
    w[jj % 16, jj // 16] = idx
    return np.tile(w, (8, 1))[:128]


def _host_prep(inputs):
    x = np.asarray(inputs["x"], np.float32)
    edge_index = np.asarray(inputs["edge_index"], np.int64)
    batch_ids = np.asarray(inputs["batch_ids"], np.int64)

    src, dst = edge_index[0], edge_index[1]
    deg = np.bincount(dst, minlength=N).astype(np.float64)
    dinv_n = np.zeros(N, np.float32)
    nz = deg > 0
    dinv_n[nz] = (1.0 / np.sqrt(deg[nz])).astype(np.float32)

    # node -> position permutation
    nodes = np.arange(N)
    pn = np.where(nodes == SWAP_A, SWAP_B, nodes)  # SWAP_B>=N so no clash
    dinv_p = np.zeros(NPAD, np.float32)
    x_p = np.zeros((NPAD, x.shape[1]), np.float32)
    batch_p = np.full(NPAD, NGR, np.int64)
    dinv_p[pn] = dinv_n
    x_p[pn] = x
    batch_p[pn] = batch_ids

    ps = np.where(src == SWAP_A, SWAP_B, src)
    pd = np.where(dst == SWAP_A, SWAP_B, dst)

    # table-row permutation within each shard: local n=c*128+p -> row p*49+c
    # (makes the exchange relay DMA per-partition contiguous)
    nl = np.arange(NPC)
    sigma = (nl % 128) * NBLK + nl // 128
    rs_ = (ps // NPC) * NPC + sigma[ps % NPC]  # src table rows

    # per core, per (dh, sh): octet arrays + per-rank scatter idx
    # core_data[r] = list over (dh, sh) of (list_of_rank_octets, list_of_rank_sidx)
    core_data = []
    for r in range(NC):
        m = (pd >= r * NPC) & (pd < (r + 1) * NPC)
        eps, epd = ps[m], pd[m] % NPC
        ers = rs_[m]
        segs = []
        for dh in range(2):
            for sh in range(2):
                mm = ((epd >= dh * DH) & (epd < (dh + 1) * DH)
                      & (eps >= sh * HALF) & (eps < (sh + 1) * HALF))
                s_ = ers[mm] - sh * HALF
                d_ = epd[mm] - dh * DH
                zr = np.int16(25087)  # zero position, relative (both halves)
                order = np.argsort(d_, kind="stable")
                s_, d_ = s_[order], d_[order]
                cnt = np.bincount(d_, minlength=DH)
                dstart = np.r_[0, np.cumsum(cnt)[:-1]]
                rank_oct, rank_sidx = [], []
                rk = 0
                while True:
                    sel = np.nonzero(cnt > 8 * rk)[0]
                    if len(sel) == 0:
                        break
                    octs = np.full((len(sel), 8), zr, np.int16)
                    for slot in range(8):
                        has = cnt[sel] > 8 * rk + slot
                        octs[has, slot] = s_[dstart[sel[has]] + 8 * rk + slot]
                    rank_oct.append(octs)
                    rank_sidx.append(sel.astype(np.int16))
                    rk += 1
                segs.append((rank_oct, rank_sidx))
        core_data.append(segs)

    # global schedule: per (seg, rank): n_oct = max over cores, rounded x16
    sched = []  # list of (seg_id, rank, n_oct)
    for seg_id in range(4):
        rmax = max(len(core_data[r][seg_id][0]) for r in range(NC))
        for rk in range(rmax):
            n = max((len(core_data[r][seg_id][0][rk])
                     if rk < len(core_data[r][seg_id][0]) else 0)
                    for r in range(NC))
            n = ((n + 15) // 16) * 16
            sched.append((seg_id, rk, n))

    # chunks: greedy within seg, splitting calls at x16 boundaries
    # chunk = (sh, [(call_id, off_oct, n_oct_sub)...])
    chunks = []
    cur = None
    cur_fill = 0
    for cid, (seg_id, rk, n) in enumerate(sched):
        sh = seg_id % 2
        off = 0
        while off < n:
            if cur is None or cur[0] != (seg_id // 2, sh) or cur_fill >= CH_OCT:
                cur = ((seg_id // 2, sh), [])
                chunks.append(cur)
                cur_fill = 0
            take = min(CH_OCT - cur_fill, n - off)
            cur[1].append((cid, off, take))
            cur_fill += take
            off += take
    return (x_p, dinv_p, batch_p, core_data, sched, chunks)


def _build(inputs, repeat=1):
    import concourse.bacc as bacc
    import concourse.mybir as mybir
    import concourse.tile as tile
    from concourse.library_config import mlp
    from concourse.bass import _add_dep_helper

    f32 = mybir.dt.float32
    bf16 = mybir.dt.bfloat16
    i16 = mybir.dt.int16

    W0 = np.asarray(inputs["W0"], np.float32)
    b0 = np.asarray(inputs["b0"], np.float32)
    W1 = np.asarray(inputs["W1"], np.float32)
    b1 = np.asarray(inputs["b1"], np.float32)
    alphas = [float(np.asarray(inputs["alpha0"]).reshape(-1)[0]),
              float(np.asarray(inputs["alpha1"]).reshape(-1)[0])]
    Wout = np.asarray(inputs["Wout"], np.float32)
    bout = float(np.asarray(inputs["bout"]).reshape(-1)[0])

    x_p, dinv_p, batch_p, core_data, sched, chunks = _host_prep(inputs)

    g0_rows = np.zeros((NPAD, 128), np.float32)
    g0_rows[:, 0:64] = x_p * dinv_p[:, None]
    nl = np.arange(NPC)
    sigma = (nl % 128) * NBLK + nl // 128
    for r in range(NC):
        blk = g0_rows[r * NPC:(r + 1) * NPC].copy()
        g0_rows[r * NPC + sigma] = blk

    # per-core blobs following sched
    per_core = []
    for r in range(NC):
        g_parts, s_parts = [], []
        for seg_id, rk, n in sched:
            ro, rs = core_data[r][seg_id]
            if rk < len(ro):
                octs, sidx = ro[rk], rs[rk]
            else:
                octs = np.zeros((0, 8), np.int16)
                sidx = np.zeros(0, np.int16)
            pad = n - len(octs)
            octs = np.vstack([octs, np.full((pad, 8), 25087, np.int16)])
            sidx = np.r_[sidx, np.full(pad, DH, np.int16)]
            g_parts.append(octs.reshape(-1))
            s_parts.append(sidx)
        gblob = np.concatenate(g_parts)
        sblob = np.concatenate(s_parts)
        sl = slice(r * NPC, (r + 1) * NPC)
        xT = np.ascontiguousarray(x_p[sl].T)
        dinvT = np.tile(dinv_p[sl][None, :], (64, 1))
        bp = batch_p[sl].reshape(NBLK, 128)
        Bnm = np.zeros((128, NBLK, 128), np.float32)
        for b in range(NBLK):
            valid = bp[b] < NGR
            Bnm[np.arange(128)[valid], b, bp[b][valid]] = 1.0
        m = dict(
            g0=g0_rows.astype(ml_dtypes.bfloat16).view(np.int16),
            gidx=_wrap16(gblob),
            sidx=_wrap16(sblob),
            xT=xT.astype(ml_dtypes.bfloat16).view(np.int16),
            dinvT=dinvT.astype(ml_dtypes.bfloat16).view(np.int16),
            Wsb=np.ascontiguousarray(
                np.stack([W0, W1]).transpose(2, 0, 1, 3).reshape(64, 512)
            ).astype(ml_dtypes.bfloat16).view(np.int16),
            Woutb=Wout.astype(ml_dtypes.bfloat16).view(np.int16),
            bb=np.stack([b0, b1], 1).astype(np.float32),
            Bnm=np.ascontiguousarray(Bnm.reshape(128, NBLK * 128)
                                     ).astype(ml_dtypes.bfloat16).view(np.int16),
        )
        per_core.append(m)

    GW = per_core[0]["gidx"].shape[1]
    SW = per_core[0]["sidx"].shape[1]

    nc = bacc.Bacc("TRN2", target_bir_lowering=False, debug=False,
                   num_devices=NC, dynamic_dma_scratch_size=32768)

    def ein(name, shape, dtype=f32):
        return nc.dram_tensor(name, shape, dtype, kind="ExternalInput")

    g0_e = ein("g0", [NPAD, 128], i16)
    gidx_e = ein("gidx", [128, GW], i16)
    sidx_e = ein("sidx", [128, SW], i16)
    xT_e = ein("xT", [64, NPC], i16)
    dinvT_e = ein("dinvT", [64, NPC], i16)
    Wsb_e = ein("Wsb", [64, 8 * 64], i16)
    Wout_e = ein("Woutb", [64, 1], i16)
    bb_e = ein("bb", [64, 2])
    Bnm_e = ein("Bnm", [128, NBLK * 128], i16)
    out_e = nc.dram_tensor("out", [NGR, 1], f32, kind="ExternalOutput")

    G_shared = nc.dram_tensor("G_shared", [NPAD, 64], f32, addr_space="Shared")
    AG_in = nc.dram_tensor("AG_in", [NPC, 64], f32)
    ar_in = nc.dram_tensor("ar_in", [1, NGR], f32)
    ar_out = nc.dram_tensor("ar_out", [1, NGR], f32, addr_space="Shared")
    RG = [list(range(NC))]

    # per-call scatter idx offsets (in octets)
    soffs = np.r_[0, np.cumsum([n for _, _, n in sched])]

    with tile.TileContext(nc) as tc:
        with (
            tc.tile_pool(name="c", bufs=1) as cpool,
            tc.tile_pool(name="w", bufs=3) as wp,
            tc.tile_pool(name="pw", bufs=4, space="PSUM") as pw,
            tc.tile_pool(name="pp", bufs=1, space="PSUM") as pp,
        ):
            lib_i = nc.gpsimd.load_library(mlp)
            regs = {}

            def reg(n):
                if n not in regs:
                    regs[n] = nc.gpsimd.to_reg(n)
                return regs[n]

            msgT = nc.alloc_sbuf_tensor("msgT", [128, CH_E], bf16)
            tab = nc.alloc_sbuf_tensor("tab", [64, DH + 1, 8], bf16)
            hT = [nc.alloc_sbuf_tensor(f"hT{k}", [64, NPC], bf16)
                  for k in range(K + 1)]
            gT = nc.alloc_sbuf_tensor("gT", [64, NPC], bf16)
            gnm = nc.alloc_sbuf_tensor("gnm", [128, NBLK, 128], bf16)
            dinvT = nc.alloc_sbuf_tensor("dinvT_sb", [64, NPC], bf16)
            gidx = nc.alloc_sbuf_tensor("gidx_sb", [128, GW], i16)
            sidx = nc.alloc_sbuf_tensor("sidx_sb", [128, SW], i16)
            Wsb = cpool.tile([64, 8, 64], bf16)
            Woutb = cpool.tile([64, 1], bf16)
            bb = cpool.tile([64, 2], f32)
            PTsb = cpool.tile([64, NGR], bf16)

            nc.sync.dma_start(out=hT[0].ap(), in_=xT_e.ap().bitcast(bf16))
            nc.sync.dma_start(out=dinvT.ap(), in_=dinvT_e.ap().bitcast(bf16))
            nc.sync.dma_start(out=gidx.ap(), in_=gidx_e[:])
            nc.sync.dma_start(out=sidx.ap(), in_=sidx_e[:])
            nc.sync.dma_start(out=Wsb[:].rearrange("p a b -> p (a b)"),
                              in_=Wsb_e.ap().bitcast(bf16))
            nc.sync.dma_start(out=Woutb[:], in_=Wout_e.ap().bitcast(bf16))
            nc.sync.dma_start(out=bb[:], in_=bb_e[:])
            nc.vector.memset(gnm.ap().rearrange("p a b -> p (a b)"), 0.0)

            def do_hop(kk, first, gather_en=True, scatter_en=True):
                src_tab = (g0_e.ap().bitcast(bf16) if first
                           else G_shared.ap().bitcast(bf16))
                # dh groups: chunks are ordered dh0 then dh1
                cur_dh = -1
                goff = 0  # edge offset into gidx blob
                for (dh, sh), parts in chunks:
                    if dh != cur_dh:
                        if cur_dh >= 0:
                            fold(cur_dh, kk)
                        nc.vector.memset(tab.ap().rearrange("p a b -> p (a b)"), 0.0)
                        cur_dh = dh
                    ne = sum(t * 8 for _, _, t in parts)
                    tab_in = (src_tab[0:HALF, :] if sh == 0
                              else src_tab[HALF:NPAD, :])
                    if gather_en:
                        gs = GSUB if GSUB else ne
                        for sub in range(0, ne, gs):
                            nsub = min(gs, ne - sub)
                            gi = nc.gpsimd.dma_gather(
                                msgT.ap()[:, sub : sub + nsub]
                                    .rearrange("p (one n) -> p one n", one=1),
                                tab_in,
                                gidx.ap()[:, (goff + sub) // 16
                                          : (goff + sub + nsub) // 16],
                                nsub, reg(nsub), 128,
                                transpose=True, single_packet=False,
                            )
                            _add_dep_helper(gi.ins, lib_i.ins, True, "lib first")
                    goff += ne
                    co = 0  # octet offset within chunk
                    for cid, off, t in parts:
                        if not scatter_en:
                            continue
                        so = soffs[cid] + off
                        sa = nc.gpsimd.scatter_add(
                            tab.ap(),
                            sidx.ap()[0:64, so // 16 : (so + t) // 16],
                            msgT.ap()[0:64, co * 8 : (co + t) * 8]
                                .rearrange("p (n d) -> p n d", d=8),
                            64, DH + 1, 8, t,
                        )
                        _add_dep_helper(sa.ins, lib_i.ins, True, "lib first")
                        co += t
                fold(cur_dh, kk)

            def fold(dh, kk):
                """tab[64, DH, 8] -> hT[kk][:, dh*DH:] = sum(slots)*dinv."""
                tv = tab.ap()[:, 0:DH, :]
                tA = msgT.ap()[0:64, 0 : DH * 4].rearrange(
                    "p (n d) -> p n d", d=4)
                tB = gT.ap()[:, 0 : DH * 2].rearrange(
                    "p (n d) -> p n d", d=2)
                tC = msgT.ap()[0:64, DH * 4 : DH * 5]
                nc.vector.tensor_tensor(out=tA, in0=tv[:, :, 0:4],
                                        in1=tv[:, :, 4:8],
                                        op=mybir.AluOpType.add)
                nc.vector.tensor_tensor(out=tB, in0=tA[:, :, 0:2],
                                        in1=tA[:, :, 2:4],
                                        op=mybir.AluOpType.add)
                nc.vector.tensor_tensor(out=tC.rearrange("p (n d) -> p n d", d=1),
                                        in0=tB[:, :, 0:1], in1=tB[:, :, 1:2],
                                        op=mybir.AluOpType.add)
                cols = slice(dh * DH, (dh + 1) * DH)
                nc.vector.tensor_tensor(out=hT[kk].ap()[:, cols], in0=tC,
                                        in1=dinvT.ap()[:, cols],
                                        op=mybir.AluOpType.mult)

            def exchange(src):
                """src [64, NPC] bf16 = g values -> AllGather into G_shared."""
                nc.sync.dma_start_transpose(gnm.ap()[:, :, 0:64], src)
                nc.sync.dma_start(
                    out=AG_in.ap().bitcast(bf16).rearrange("(p c) f -> p c f", p=128),
                    in_=gnm.ap(),
                )
                nc.gpsimd.collective_compute(
                    "AllGather", mybir.AluOpType.bypass,
                    replica_groups=RG, ins=[AG_in[:]], outs=[G_shared[:]],
                )

            def wphase(layer):
                for c in range(13):
                    c0 = c * 512
                    cw = min(512, NPC - c0)
                    ps = pw.tile([64, 512], f32)
                    for k in range(K + 1):
                        nc.tensor.matmul(
                            ps[:, 0:cw], lhsT=Wsb[:, layer * 4 + k, :],
                            rhs=hT[k].ap()[:, c0 : c0 + cw],
                            start=(k == 0), stop=(k == K),
                        )
                    nc.scalar.activation(
                        hT[0].ap()[:, c0 : c0 + cw], ps[:, 0:cw],
                        mybir.ActivationFunctionType.Prelu,
                        bias=bb[:, layer : layer + 1], scale=1.0,
                        alpha=alphas[layer],
                    )

            def dbg_out(src_bf16_col):
                d = wp.tile([64, 1], f32, tag="dbg")
                nc.vector.tensor_copy(out=d[:], in_=src_bf16_col)
                nc.sync.dma_start(out=out_e[0:64, :], in_=d[:])

            if _V2T == "hops8":
                for _ in range(8):
                    do_hop(1, True)
                dbg_out(hT[1].ap()[:, 0:1])
            elif _V2T == "ag8":
                for _ in range(8):
                    exchange(gT.ap()[:, :])
                dbg_out(hT[1].ap()[:, 0:1])
            elif _V2T == "wp8":
                for _ in range(8):
                    wphase(0)
                dbg_out(hT[0].ap()[:, 0:1])
            elif _V2T == "hop":
                do_hop(1, True)
                dbg_out(hT[1].ap()[:, 0:1])
            elif _V2T == "gonly":
                do_hop(1, True, scatter_en=False)
                dbg_out(hT[1].ap()[:, 0:1])
            elif _V2T == "sonly":
                do_hop(1, True, gather_en=False)
                dbg_out(hT[1].ap()[:, 0:1])
            elif _V2T == "hopx":
                do_hop(1, True)
                nc.vector.tensor_tensor(out=gT.ap()[:, :], in0=hT[1].ap()[:, :],
                                        in1=dinvT.ap()[:, :],
                                        op=mybir.AluOpType.mult)
                exchange(gT.ap()[:, :])
                do_hop(2, False)
                dbg_out(hT[2].ap()[:, 0:1])
            elif _V2T == "wp0":
                do_hop(1, True)
                wphase(0)
                dbg_out(hT[0].ap()[:, 0:1])
            for _rep in range(repeat if _V2T == "full" else 0):
                for layer in range(2):
                    first = layer == 0
                    for k in range(1, K + 1):
                        do_hop(k, first and k == 1)
                        if k < K:
                            gcols = gT.ap()[:, :]
                            nc.vector.tensor_tensor(
                                out=gcols, in0=hT[k].ap()[:, :],
                                in1=dinvT.ap()[:, :], op=mybir.AluOpType.mult)
                            exchange(gcols)
                    wphase(layer)
                    if layer == 0:
                        nc.vector.tensor_tensor(
                            out=gT.ap()[:, :], in0=hT[0].ap()[:, :],
                            in1=dinvT.ap()[:, :], op=mybir.AluOpType.mult)
                        exchange(gT.ap()[:, :])

                # pooling: h1 = hT[0]
                Bnm = msgT.ap()[:, 0 : NBLK * 128].rearrange(
                    "p (a b) -> p a b", b=128)
                nc.sync.dma_start(out=msgT.ap()[:, 0 : NBLK * 128],
                                  in_=Bnm_e.ap().bitcast(bf16))
                h1nm = msgT.ap()[:, NBLK * 128 : NBLK * 192].rearrange(
                    "p (a b) -> p a b", b=64)
                nc.sync.dma_start_transpose(h1nm, hT[0].ap()[:, :])
                PT = pp.tile([64, NGR], f32)
                for b in range(NBLK):
                    nc.tensor.matmul(PT[:], lhsT=h1nm[:, b, :], rhs=Bnm[:, b, :],
                                     start=(b == 0), stop=(b == NBLK - 1))
                nc.vector.tensor_copy(out=PTsb[:], in_=PT[:])
                zps = pp.tile([1, NGR], f32, tag="zps")
                nc.tensor.matmul(zps[:], lhsT=Woutb[:], rhs=PTsb[:],
                                 start=True, stop=True)
                zsb = wp.tile([1, NGR], f32, tag="zsb")
                nc.vector.tensor_copy(out=zsb[:], in_=zps[:])
                nc.sync.dma_start(out=ar_in[:], in_=zsb[:])
                nc.gpsimd.collective_compute(
                    "AllReduce", mybir.AluOpType.add,
                    replica_groups=RG, ins=[ar_in[:]], outs=[ar_out[:]],
                )
                res = wp.tile([1, NGR], f32, tag="res")
                nc.sync.dma_start(out=res[:], in_=ar_out[:])
                nc.vector.tensor_scalar_add(res[:], res[:], float(bout))
                nc.sync.dma_start(out=out_e.ap().rearrange("g one -> one g"),
                                  in_=res[:])

    nc.compile()
    return nc, per_core


def kernel(**inputs):
    from concourse.bass_utils import run_bass_kernel_spmd

    nc, per_core = _build(inputs, repeat=1)
    results = run_bass_kernel_spmd(nc, per_core, list(range(8)))
    return results.results[0]["out"].astype(np.float32)


def estimate_hw_time_ns(inputs, r_hi=3, n_rep=8):
    import time
    from concourse.bass_utils import run_bass_kernel_spmd

    walls = {}
    for r in (1, r_hi):
        nc, per_core = _build(inputs, repeat=r)
        run_bass_kernel_spmd(nc, per_core, list(range(8)))  # warm
        ws = []
        for _ in range(n_rep):
            t0 = time.time()
            run_bass_kernel_spmd(nc, per_core, list(range(8)))
            ws.append(time.time() - t0)
        walls[r] = min(ws)
    return (walls[r_hi] - walls[1]) / (r_hi - 1) * 1e9


if __name__ == "__main__":
    import jax
    import reference

    cpu = jax.devices("cpu")[0]
    with jax.default_device(cpu):
        ins = {k: np.asarray(v) for k, v in reference.setup_inputs().items()}
        exp = np.asarray(reference.reference(**ins))
    got = kernel(**ins)
    err = np.abs(got - exp).max() / (np.abs(exp).max() + 1e-12)
    print("rel err:", err)
